# revision 32
# baseline (speedup 1.0000x reference)
"""MLA attention kernel for 8 Trainium2 NeuronCores.

Sharding: core i -> batch b = i//4, head group hg = i%4 (32 heads each).
Latent down-projections replicated within a batch group; Wq_up/Wq_rope/
Wk_up/Wv_up/Wo sharded by head.  Host sums the 4 partial outputs per batch.

Device program (identical on all cores, SPMD over different data):
  - all matmuls bf16 with fp32 PSUM accumulation
  - projections computed feature-major (features on partitions) so that
    attention scores S^T[k, q] = kT.T @ qT need no transposes
  - softmax: exp on ScalarE (scale 1/sqrt(96) folded in, no max subtraction:
    scores are ~N(0,1)), denominator via an appended ones-column of V in the
    attn@V matmul, division via DVE reciprocal_approx_fast + gpsimd
    partition broadcast
  - startup: first two Wq_down m-slabs stream in, then x; the down-proj
    runs k-OUTER over 4 PSUM accumulators so the PE consumes x chunks as
    the 8MB DMA delivers them (instead of stalling ~38us for all of x)
  - the attention exp chains are ACT-throughput bound (8 exps x 687ns vs
    3.4us of PE work per (head,qc) chain); proj MMs for group g+1 are
    emitted INTERLEAVED between attn(g)'s score/av matmuls so the strict
    in-order PE queue always has independent work while exps drain
"""

import sys

sys.path.insert(0, "/opt/trn_rl_repo")

import numpy as np
import ml_dtypes

import concourse.bass as bass
import concourse.tile as tile
from concourse import bacc, mybir
from concourse.bass_utils import run_bass_kernel_spmd

P = 128
T = 1024          # tokens per batch
DM = 4096         # d_model
KX = DM // P      # 32 feature chunks of x
LAT = 512         # latent dim
LC = LAT // P     # 4 latent chunks
NHC = 32          # heads per core
DH = 32           # head dim (compressed part)
DR = 64           # rope dim per head
NB = 2            # batch
SCALE = 1.0 / float(np.sqrt(DH + DR))

BF = mybir.dt.bfloat16
F32 = mybir.dt.float32

_CACHE = {}


def _build_program():
    nc = bacc.Bacc("TRN2", target_bir_lowering=False, num_devices=8)

    xT = nc.declare_dram_parameter("xT", [DM, T], BF, isOutput=False)
    wqd = nc.declare_dram_parameter("wqd", [DM, LAT], BF, isOutput=False)
    wkvd = nc.declare_dram_parameter("wkvd", [DM, LAT], BF, isOutput=False)
    wqu = nc.declare_dram_parameter("wqu", [LAT, NHC * DH], BF, isOutput=False)
    wku = nc.declare_dram_parameter("wku", [LAT, NHC * DH], BF, isOutput=False)
    wvu = nc.declare_dram_parameter("wvu", [LAT, NHC * DH], BF, isOutput=False)
    wqr = nc.declare_dram_parameter("wqr", [DM, NHC * DR], BF, isOutput=False)
    wkr = nc.declare_dram_parameter("wkr", [DM, DR], BF, isOutput=False)
    wo = nc.declare_dram_parameter("wo", [NHC * DH, DM], BF, isOutput=False)
    out = nc.declare_dram_parameter("out", [T, DM], F32, isOutput=True)

    from contextlib import ExitStack

    with tile.TileContext(nc) as tc, ExitStack() as octx:
        const = octx.enter_context(tc.tile_pool(name="const", bufs=1))

        xT_sb = const.tile([P, KX, T], BF, name="xT_sb")
        xT_r = xT[:].rearrange("(ko p) t -> p ko t", p=P)
        wkr_sb = const.tile([P, KX, DR], BF, name="wkr_sb")
        wvu_sb = const.tile([P, LC, NHC * DH], BF, name="wvu_sb")

        cq_sb = const.tile([P, LC, T], BF, name="cq_sb")      # c_q^T
        ckv_sb = const.tile([P, LC, T], BF, name="ckv_sb")    # c_kv^T
        kr_sb = const.tile([DR, T], BF, name="kr_sb")         # k_rope^T (shared)
        # v token-major, per (key-chunk, head): cols 0:32 = v, col 32 = ones
        v_sb = const.tile([P, 8, NHC, 34], BF, name="v_sb")
        # attention output, feature-major: head h -> [32*(h%4):.., h//4, :]
        aout_sb = const.tile([P, 8, T], BF, name="aout_sb")
        # prefetch buffer for the first half of Wo slab n=0 (removes the
        # phase-E start stall; the rest double-buffers under E compute)
        wos0_sb = const.tile([P, 2, 512], BF, name="wos0_sb")
        # softmax denominators, spread across partitions {0,32,64,96} of two
        # tiles (DVE writes must be 32-partition-aligned); persistent+memset
        # so the batched reciprocal never reads uninitialized rows
        dens = [const.tile([P, 512], F32, name=f"den{i}") for i in range(2)]
        recs = [const.tile([P, 512], F32, name=f"rec{i}") for i in range(2)]

        nc.vector.memset(v_sb[:, :, :, 32:33], 1.0)
        for i in range(2):
            nc.vector.memset(dens[i][:], 1.0)

        with ExitStack() as ctx:
            wpool = ctx.enter_context(tc.tile_pool(name="wpool", bufs=3))
            cpp = ctx.enter_context(tc.tile_pool(name="cpp", bufs=4, space="PSUM"))
            qkpool = ctx.enter_context(tc.tile_pool(name="qkpool", bufs=12))
            ppool = ctx.enter_context(tc.tile_pool(name="ppool", bufs=2))
            spp = ctx.enter_context(tc.tile_pool(name="spp", bufs=2, space="PSUM"))
            avp = ctx.enter_context(tc.tile_pool(name="avp", bufs=2, space="PSUM"))
            rrpool = ctx.enter_context(tc.tile_pool(name="rrpool", bufs=1))

            # ---- DMA order: first two wqd slabs, then x (8 fine chunks so
            # the k-outer loop can start early), then wkr/wvu.
            def load_bslab(m):
                ws = wpool.tile([P, KX, P], BF, tag="wqrs", name=f"bq{m}")
                b_src = wqd[:, m * P:(m + 1) * P].rearrange(
                    "(ko p) m -> p ko m", p=P
                )
                nc.sync.dma_start(out=ws[:, 0:16, :], in_=b_src[:, 0:16, :])
                nc.sync.dma_start(out=ws[:, 16:KX, :], in_=b_src[:, 16:KX, :])
                return ws

            def load_x_quad(i):
                nc.sync.dma_start(
                    out=xT_sb[:, i * 4:(i + 1) * 4, :],
                    in_=xT_r[:, i * 4:(i + 1) * 4, :],
                )

            # interleave weight-slab and x DMAs so neither gates the other
            bslab01 = [load_bslab(0)]
            load_x_quad(0)
            bslab01.append(load_bslab(1))
            for i in range(1, 8):
                load_x_quad(i)
            nc.sync.dma_start(
                out=wkr_sb[:], in_=wkr[:].rearrange("(ko p) d -> p ko d", p=P)
            )
            nc.sync.dma_start(
                out=wvu_sb[:], in_=wvu[:].rearrange("(c p) m -> p c m", p=P)
            )

            # ---- Phase B1: wqd m0/m1, k-OUTER over 4 accumulators so the
            # PE consumes x chunks at DMA pace instead of stalling; blocks
            # of 4 k alternating m so the m1 chains don't head-of-line block
            # before the m1 slab DMA lands.
            ps4 = [
                cpp.tile([P, 512], F32, tag="cps", name=f"b01_{i}")
                for i in range(4)
            ]
            for kb in range(8):
                for m in range(2):
                    for k in range(kb * 4, kb * 4 + 4):
                        for hf in range(2):
                            nc.tensor.matmul(
                                ps4[2 * m + hf][:],
                                bslab01[m][:, k, :],
                                xT_sb[:, k, hf * 512:(hf + 1) * 512],
                                start=(k == 0),
                                stop=(k == KX - 1),
                            )
            for m in range(2):
                for hf in range(2):
                    nc.vector.tensor_copy(
                        out=cq_sb[:, m, hf * 512:(hf + 1) * 512],
                        in_=ps4[2 * m + hf][:],
                    )

            # ---- Phase B2: wqd m2/m3 + wkvd (k-inner; x resident by now) ----
            for wd, cdst, ms in (
                (wqd, cq_sb, (2, 3)),
                (wkvd, ckv_sb, (0, 1, 2, 3)),
            ):
                for m in ms:
                    wslab = wpool.tile([P, KX, P], BF, tag="wqrs", name="bslab")
                    b_src = wd[:, m * P:(m + 1) * P].rearrange(
                        "(ko p) m -> p ko m", p=P
                    )
                    nc.sync.dma_start(out=wslab[:, 0:16, :], in_=b_src[:, 0:16, :])
                    nc.sync.dma_start(out=wslab[:, 16:KX, :], in_=b_src[:, 16:KX, :])
                    for hf in range(2):
                        ps = cpp.tile([P, 512], F32, tag="cps")
                        for k in range(KX):
                            nc.tensor.matmul(
                                ps[:],
                                wslab[:, k, :],
                                xT_sb[:, k, hf * 512:(hf + 1) * 512],
                                start=(k == 0),
                                stop=(k == KX - 1),
                            )
                        nc.vector.tensor_copy(
                            out=cdst[:, m, hf * 512:(hf + 1) * 512], in_=ps[:]
                        )

            # k_rope^T [64, T]
            for hf in range(2):
                ps = cpp.tile([P, 512], F32, tag="cps")
                for k in range(KX):
                    nc.tensor.matmul(
                        ps[:DR, :],
                        wkr_sb[:, k, :],
                        xT_sb[:, k, hf * 512:(hf + 1) * 512],
                        start=(k == 0),
                        stop=(k == KX - 1),
                    )
                nc.vector.tensor_copy(
                    out=kr_sb[:, hf * 512:(hf + 1) * 512], in_=ps[:DR, :]
                )

            # ---- Phase V: v = c_kv @ Wv_up (token-major), interleaved heads ----
            for tt in range(8):
                for hf in range(2):
                    ps = cpp.tile([P, 512], F32, tag="cps")
                    for lc in range(LC):
                        nc.tensor.matmul(
                            ps[:],
                            ckv_sb[:, lc, tt * P:(tt + 1) * P],
                            wvu_sb[:, lc, hf * 512:(hf + 1) * 512],
                            start=(lc == 0),
                            stop=(lc == LC - 1),
                        )
                    nc.vector.tensor_copy(
                        out=v_sb[:, tt, hf * 16:(hf + 1) * 16, 0:32],
                        in_=ps[:].rearrange("p (h d) -> p h d", h=16),
                    )

            # ---- projections for one head group, as a list of small emission
            # steps so they can be interleaved into the previous group's
            # attention (keeps the in-order PE queue fed while exps drain).
            def proj_steps(g):
                qt = [
                    qkpool.tile([P, T], BF, tag="qkt", name=f"qt{g}_{j}")
                    for j in range(4)
                ]
                kt = [
                    qkpool.tile([P, T], BF, tag="qkt", name=f"kt{g}_{j}")
                    for j in range(4)
                ]
                steps = []
                state = {}

                # up-projections (wqu -> qt rows 64:96, wku -> kt rows 64:96)
                def up_dma(wu, key):
                    def f():
                        ws = wpool.tile([P, LC, P], BF, tag="wups", name=f"up{key}")
                        nc.sync.dma_start(
                            out=ws[:],
                            in_=wu[:, g * P:(g + 1) * P].rearrange(
                                "(c p) m -> p c m", p=P
                            ),
                        )
                        state[key] = ws
                    return f

                def up_mm(key, src, hf, lc):
                    def f():
                        if lc == 0:
                            state[(key, hf)] = cpp.tile(
                                [P, 512], F32, tag="cps", name=f"up_ps_{key}"
                            )
                        nc.tensor.matmul(
                            state[(key, hf)][:],
                            state[key][:, lc, :],
                            src[:, lc, hf * 512:(hf + 1) * 512],
                            start=(lc == 0),
                            stop=(lc == LC - 1),
                        )
                    return f

                def up_cast(key, dst, hf):
                    def f():
                        ps = state[(key, hf)]
                        sl = slice(hf * 512, (hf + 1) * 512)
                        for j in range(4):
                            nc.vector.tensor_copy(
                                out=dst[j][DR:DR + DH, sl],
                                in_=ps[j * DH:(j + 1) * DH, :],
                            )
                    return f

                for wu, key, src, dst in (
                    (wqu, "q", cq_sb, qt),
                    (wku, "k", ckv_sb, kt),
                ):
                    steps.append(up_dma(wu, key))
                    for hf in range(2):
                        for lc in range(LC):
                            steps.append(up_mm(key, src, hf, lc))
                        steps.append(up_cast(key, dst, hf))

                # shared k_rope rows into each kt (DVE, no MM deps)
                for j in range(4):
                    steps.append(
                        lambda j=j: nc.vector.tensor_copy(
                            out=kt[j][0:DR, :], in_=kr_sb[:]
                        )
                    )

                # q_rope slabs: slab s covers heads 2s, 2s+1 (rows 0:64)
                def qr_dma(s):
                    def f():
                        ws = wpool.tile([P, KX, P], BF, tag="wqrs", name=f"qr{s}")
                        src = wqr[:, (2 * g + s) * P:(2 * g + s + 1) * P].rearrange(
                            "(ko p) m -> p ko m", p=P
                        )
                        nc.sync.dma_start(out=ws[:, 0:16, :], in_=src[:, 0:16, :])
                        nc.sync.dma_start(out=ws[:, 16:KX, :], in_=src[:, 16:KX, :])
                        state[("qr", s)] = ws
                    return f

                def qr_mm(s, hf, k):
                    def f():
                        if k == 0:
                            state[("qrps", s, hf)] = cpp.tile(
                                [P, 512], F32, tag="cps", name="qr_ps"
                            )
                        nc.tensor.matmul(
                            state[("qrps", s, hf)][:],
                            state[("qr", s)][:, k, :],
                            xT_sb[:, k, hf * 512:(hf + 1) * 512],
                            start=(k == 0),
                            stop=(k == KX - 1),
                        )
                    return f

                def qr_cast(s, hf):
                    def f():
                        ps = state[("qrps", s, hf)]
                        sl = slice(hf * 512, (hf + 1) * 512)
                        nc.vector.tensor_copy(out=qt[2 * s][0:DR, sl], in_=ps[0:DR, :])
                        nc.vector.tensor_copy(
                            out=qt[2 * s + 1][0:DR, sl], in_=ps[DR:P, :]
                        )
                    return f

                for s in range(2):
                    steps.append(qr_dma(s))
                    for hf in range(2):
                        for k in range(KX):
                            steps.append(qr_mm(s, hf, k))
                        steps.append(qr_cast(s, hf))
                return qt, kt, steps

            # ---- attention for group g, with proj(g+1) steps interleaved ----
            def emit_attn(g, qt, kt, steps):
                si = 0
                nsteps = len(steps)

                def fill(n, limit):
                    nonlocal si
                    end = min(limit, si + n, nsteps)
                    while si < end:
                        steps[si]()
                        si += 1

                chains = [(j, qc) for qc in range(2) for j in range(4)]
                nch = len(chains)
                # per-group batched softmax denominators: DVE reciprocal cost
                # scales with per-partition elements, so spread the 8 ones-row
                # results across partitions and run 2 reciprocals per group
                # instead of 8 single-partition ones (8x less DVE time).
                avs_list = []
                for ci, (j, qc) in enumerate(chains):
                    limit = (nsteps * (ci + 1) + nch - 1) // nch
                    h = 4 * g + j
                    qsl = slice(qc * 512, (qc + 1) * 512)
                    probs = ppool.tile([P, 8, 512], BF, tag="probs", name="probs")
                    for kc in range(8):
                        sp = spp.tile([P, 512], F32, tag="sps", name="sps")
                        nc.tensor.matmul(
                            sp[:],
                            kt[j][0:96, kc * P:(kc + 1) * P],
                            qt[j][0:96, qsl],
                            start=True,
                            stop=True,
                        )
                        nc.scalar.activation(
                            out=probs[:, kc, :],
                            in_=sp[:],
                            func=mybir.ActivationFunctionType.Exp,
                            scale=SCALE,
                        )
                        if kc >= 1:
                            fill(3, limit)
                    av = avp.tile([33, 512], F32, tag="avp", name="av")
                    for kc in range(8):
                        nc.tensor.matmul(
                            av[:],
                            v_sb[:, kc, h, 0:33],
                            probs[:, kc, :],
                            start=(kc == 0),
                            stop=(kc == 7),
                        )
                        if kc % 2 == 1:
                            fill(2, limit)
                    # stage unnormalized values straight into aout (bf16),
                    # normalize in place after the group-wide reciprocal
                    nc.vector.tensor_copy(
                        out=aout_sb[j * DH:(j + 1) * DH, g, qsl],
                        in_=av[0:DH, :],
                    )
                    dp = (ci % 4) * 32
                    nc.vector.tensor_copy(
                        out=dens[ci // 4][dp:dp + 1, :], in_=av[32:33, :]
                    )
                    avs_list.append((j, qsl))
                    fill(nsteps, limit)
                for i in range(2):
                    nc.vector.reciprocal(recs[i][:], dens[i][:])
                for ci, (j, qsl) in enumerate(avs_list):
                    # rrep band matches aout's base partition (TensorTensor
                    # requires equal SBUF base partitions on both inputs)
                    rrep = rrpool.tile([P, 512], BF, tag="rr", name="rrep")
                    dp = (ci % 4) * 32
                    jb = j * DH
                    # partition_broadcast replicates PARTITION 0 of its input
                    # and writes from partition 0 — both APs' bases are
                    # ignored — so stage the reciprocal row to partition 0
                    # (bf16: cheaper broadcast, precision is bf16 anyway)
                    rst = rrpool.tile([1, 512], BF, tag="rst", name="rst")
                    nc.vector.tensor_copy(
                        out=rst[:], in_=recs[ci // 4][dp:dp + 1, :]
                    )
                    nc.gpsimd.partition_broadcast(rrep[:], rst[:])
                    nc.vector.tensor_mul(
                        out=aout_sb[jb:jb + DH, g, qsl],
                        in0=aout_sb[jb:jb + DH, g, qsl],
                        in1=rrep[jb:jb + DH, :],
                    )
                fill(nsteps, nsteps)

            qt, kt, steps0 = proj_steps(0)
            for st in steps0:
                st()
            for g in range(8):
                if g < 7:
                    nqt, nkt, steps = proj_steps(g + 1)
                else:
                    steps = []
                if g == 6:
                    nc.sync.dma_start(
                        out=wos0_sb[:],
                        in_=wo[:, 0:512].rearrange("(kc p) m -> p kc m", p=P)[
                            :, 0:2, :
                        ],
                    )
                emit_attn(g, qt, kt, steps)
                if g < 7:
                    qt, kt = nqt, nkt

        # ---- Phase E: out = aout^T @ Wo  (token-major), Wo streamed once ----
        with ExitStack() as ctx:
            wop = ctx.enter_context(tc.tile_pool(name="wop", bufs=2))
            epp = ctx.enter_context(tc.tile_pool(name="epp", bufs=8, space="PSUM"))
            eop = ctx.enter_context(tc.tile_pool(name="eop", bufs=2))
            for n in range(8):
                wo_src = wo[:, n * 512:(n + 1) * 512].rearrange(
                    "(kc p) m -> p kc m", p=P
                )
                woslab = wop.tile([P, 8, 512], BF, tag="wos")
                if n == 0:
                    # first two kc chunks prefetched during group-6 attention
                    nc.sync.dma_start(out=woslab[:, 2:8, :], in_=wo_src[:, 2:8, :])
                else:
                    nc.sync.dma_start(out=woslab[:, 0:4, :], in_=wo_src[:, 0:4, :])
                    nc.sync.dma_start(out=woslab[:, 4:8, :], in_=wo_src[:, 4:8, :])
                pss = [
                    epp.tile([P, 512], F32, tag="eps", name=f"eps_{n}_{i}")
                    for i in range(8)
                ]
                # batched output tile: one store DMA per n instead of 8
                # (the Sync engine's ~1us per descriptor was pacing phase E);
                # per-tt copies right after each tt's last matmul so PSUM
                # slots free progressively before n+1 starts
                eot = eop.tile([P, 8, 512], F32, tag="eot")
                for kc in range(8):
                    if n == 0 and kc < 2:
                        src = wos0_sb[:, kc, :]
                    else:
                        src = woslab[:, kc, :]
                    for tt in range(8):
                        nc.tensor.matmul(
                            pss[tt][:],
                            aout_sb[:, kc, tt * P:(tt + 1) * P],
                            src,
                            start=(kc == 0),
                            stop=(kc == 7),
                        )
                        if kc == 7:
                            nc.vector.tensor_copy(
                                out=eot[:, tt, :], in_=pss[tt][:]
                            )
                nc.sync.dma_start(
                    out=out[:, n * 512:(n + 1) * 512].rearrange(
                        "(tt p) m -> p tt m", p=P
                    ),
                    in_=eot[:],
                )

    nc.compile()
    return nc


def _prep_inputs(inputs):
    bf = ml_dtypes.bfloat16
    x = np.asarray(inputs["x"], dtype=np.float32)
    Wq_down = np.asarray(inputs["Wq_down"], dtype=np.float32).astype(bf)
    Wkv_down = np.asarray(inputs["Wkv_down"], dtype=np.float32).astype(bf)
    Wq_up = np.asarray(inputs["Wq_up"], dtype=np.float32).astype(bf)
    Wk_up = np.asarray(inputs["Wk_up"], dtype=np.float32).astype(bf)
    Wv_up = np.asarray(inputs["Wv_up"], dtype=np.float32).astype(bf)
    Wq_rope = np.asarray(inputs["Wq_rope"], dtype=np.float32).astype(bf)
    Wk_rope = np.asarray(inputs["Wk_rope"], dtype=np.float32).astype(bf)
    Wo = np.asarray(inputs["Wo"], dtype=np.float32).astype(bf)

    xT = [np.ascontiguousarray(x[b].T).astype(bf) for b in range(NB)]

    in_maps = []
    for core in range(8):
        b = core // 4
        hg = core % 4
        hs = slice(hg * NHC * DH, (hg + 1) * NHC * DH)        # head-dim cols
        rs = slice(hg * NHC * DR, (hg + 1) * NHC * DR)        # rope cols
        in_maps.append(
            {
                "xT": xT[b],
                "wqd": Wq_down,
                "wkvd": Wkv_down,
                "wqu": np.ascontiguousarray(Wq_up[:, hs]),
                "wku": np.ascontiguousarray(Wk_up[:, hs]),
                "wvu": np.ascontiguousarray(Wv_up[:, hs]),
                "wqr": np.ascontiguousarray(Wq_rope[:, rs]),
                "wkr": Wk_rope,
                "wo": np.ascontiguousarray(Wo[hs, :]),
            }
        )
    return in_maps


def kernel(**inputs):
    if "nc" not in _CACHE:
        _CACHE["nc"] = _build_program()
    nc = _CACHE["nc"]
    in_maps = _prep_inputs(inputs)
    res = run_bass_kernel_spmd(nc, in_maps, list(range(8)))
    out = np.zeros((NB, T, DM), dtype=np.float32)
    for core in range(8):
        out[core // 4] += res.results[core]["out"]
    return out


# revision 34
# speedup vs baseline: 1.0347x; 1.0347x over previous
"""MLA attention kernel for 8 Trainium2 NeuronCores.

Sharding: core i -> batch b = i//4, head group hg = i%4 (32 heads each).
Latent down-projections replicated within a batch group; Wq_up/Wq_rope/
Wk_up/Wv_up/Wo sharded by head.  Host sums the 4 partial outputs per batch.

Device program (identical on all cores, SPMD over different data):
  - all matmuls bf16 with fp32 PSUM accumulation
  - projections computed feature-major (features on partitions) so that
    attention scores S^T[k, q] = kT.T @ qT need no transposes
  - softmax: exp on ScalarE (scale 1/sqrt(96) folded in, no max subtraction:
    scores are ~N(0,1)), denominator via an appended ones-column of V in the
    attn@V matmul, division via DVE reciprocal_approx_fast + gpsimd
    partition broadcast
  - startup: first two Wq_down m-slabs stream in, then x; the down-proj
    runs k-OUTER over 4 PSUM accumulators so the PE consumes x chunks as
    the 8MB DMA delivers them (instead of stalling ~38us for all of x)
  - the attention exp chains are ACT-throughput bound (8 exps x 687ns vs
    3.4us of PE work per (head,qc) chain); proj MMs for group g+1 are
    emitted INTERLEAVED between attn(g)'s score/av matmuls so the strict
    in-order PE queue always has independent work while exps drain
"""

import sys

sys.path.insert(0, "/opt/trn_rl_repo")

import numpy as np
import ml_dtypes

import concourse.bass as bass
import concourse.tile as tile
from concourse import bacc, mybir
from concourse.bass_utils import run_bass_kernel_spmd

P = 128
T = 1024          # tokens per batch
DM = 4096         # d_model
KX = DM // P      # 32 feature chunks of x
LAT = 512         # latent dim
LC = LAT // P     # 4 latent chunks
NHC = 32          # heads per core
DH = 32           # head dim (compressed part)
DR = 64           # rope dim per head
NB = 2            # batch
SCALE = 1.0 / float(np.sqrt(DH + DR))

BF = mybir.dt.bfloat16
F32 = mybir.dt.float32

_CACHE = {}


def _build_program():
    nc = bacc.Bacc("TRN2", target_bir_lowering=False, num_devices=8)

    xT = nc.declare_dram_parameter("xT", [DM, T], BF, isOutput=False)
    wqd = nc.declare_dram_parameter("wqd", [DM, LAT], BF, isOutput=False)
    wkvd = nc.declare_dram_parameter("wkvd", [DM, LAT], BF, isOutput=False)
    wqu = nc.declare_dram_parameter("wqu", [LAT, NHC * DH], BF, isOutput=False)
    wku = nc.declare_dram_parameter("wku", [LAT, NHC * DH], BF, isOutput=False)
    wvu = nc.declare_dram_parameter("wvu", [LAT, NHC * DH], BF, isOutput=False)
    wqr = nc.declare_dram_parameter("wqr", [DM, NHC * DR], BF, isOutput=False)
    wkr = nc.declare_dram_parameter("wkr", [DM, DR], BF, isOutput=False)
    wo = nc.declare_dram_parameter("wo", [NHC * DH, DM], BF, isOutput=False)
    out = nc.declare_dram_parameter("out", [T, DM], F32, isOutput=True)

    from contextlib import ExitStack

    with tile.TileContext(nc) as tc, ExitStack() as octx:
        const = octx.enter_context(tc.tile_pool(name="const", bufs=1))

        xT_sb = const.tile([P, KX, T], BF, name="xT_sb")
        xT_r = xT[:].rearrange("(ko p) t -> p ko t", p=P)
        wkr_sb = const.tile([P, KX, DR], BF, name="wkr_sb")
        wvu_sb = const.tile([P, LC, NHC * DH], BF, name="wvu_sb")

        cq_sb = const.tile([P, LC, T], BF, name="cq_sb")      # c_q^T
        ckv_sb = const.tile([P, LC, T], BF, name="ckv_sb")    # c_kv^T
        kr_sb = const.tile([DR, T], BF, name="kr_sb")         # k_rope^T (shared)
        # v token-major, per (key-chunk, head): cols 0:32 = v, col 32 = ones
        v_sb = const.tile([P, 8, NHC, 34], BF, name="v_sb")
        # attention output, feature-major: head h -> [32*(h%4):.., h//4, :]
        aout_sb = const.tile([P, 8, T], BF, name="aout_sb")
        # prefetch buffer for the first half of Wo slab n=0 (removes the
        # phase-E start stall; the rest double-buffers under E compute)
        wos0_sb = const.tile([P, 2, 512], BF, name="wos0_sb")
        # softmax denominators, spread across partitions {0,32,64,96} of two
        # tiles (DVE writes must be 32-partition-aligned); persistent+memset
        # so the batched reciprocal never reads uninitialized rows
        dens = [const.tile([P, 512], F32, name=f"den{i}") for i in range(2)]
        recs = [const.tile([P, 512], F32, name=f"rec{i}") for i in range(2)]

        nc.vector.memset(v_sb[:, :, :, 32:33], 1.0)
        for i in range(2):
            nc.vector.memset(dens[i][:], 1.0)

        with ExitStack() as ctx:
            wpool = ctx.enter_context(tc.tile_pool(name="wpool", bufs=3))
            cpp = ctx.enter_context(tc.tile_pool(name="cpp", bufs=4, space="PSUM"))
            qkpool = ctx.enter_context(tc.tile_pool(name="qkpool", bufs=12))
            ppool = ctx.enter_context(tc.tile_pool(name="ppool", bufs=2))
            spp = ctx.enter_context(tc.tile_pool(name="spp", bufs=2, space="PSUM"))
            avp = ctx.enter_context(tc.tile_pool(name="avp", bufs=2, space="PSUM"))
            rrpool = ctx.enter_context(tc.tile_pool(name="rrpool", bufs=1))

            # ---- DMA order: first two wqd slabs, then x (8 fine chunks so
            # the k-outer loop can start early), then wkr/wvu.
            def load_bslab(m):
                ws = wpool.tile([P, KX, P], BF, tag="wqrs", name=f"bq{m}")
                b_src = wqd[:, m * P:(m + 1) * P].rearrange(
                    "(ko p) m -> p ko m", p=P
                )
                nc.sync.dma_start(out=ws[:, 0:16, :], in_=b_src[:, 0:16, :])
                nc.sync.dma_start(out=ws[:, 16:KX, :], in_=b_src[:, 16:KX, :])
                return ws

            def load_x_quad(i):
                nc.sync.dma_start(
                    out=xT_sb[:, i * 4:(i + 1) * 4, :],
                    in_=xT_r[:, i * 4:(i + 1) * 4, :],
                )

            # interleave weight-slab and x DMAs so neither gates the other
            bslab01 = [load_bslab(0)]
            load_x_quad(0)
            bslab01.append(load_bslab(1))
            for i in range(1, 8):
                load_x_quad(i)
            nc.sync.dma_start(
                out=wkr_sb[:], in_=wkr[:].rearrange("(ko p) d -> p ko d", p=P)
            )
            nc.sync.dma_start(
                out=wvu_sb[:], in_=wvu[:].rearrange("(c p) m -> p c m", p=P)
            )

            # ---- Phase B1: wqd m0/m1, k-OUTER over 4 accumulators so the
            # PE consumes x chunks at DMA pace instead of stalling; blocks
            # of 4 k alternating m so the m1 chains don't head-of-line block
            # before the m1 slab DMA lands.
            ps4 = [
                cpp.tile([P, 512], F32, tag="cps", name=f"b01_{i}")
                for i in range(4)
            ]
            for kb in range(8):
                for m in range(2):
                    for k in range(kb * 4, kb * 4 + 4):
                        for hf in range(2):
                            nc.tensor.matmul(
                                ps4[2 * m + hf][:],
                                bslab01[m][:, k, :],
                                xT_sb[:, k, hf * 512:(hf + 1) * 512],
                                start=(k == 0),
                                stop=(k == KX - 1),
                            )
            for m in range(2):
                for hf in range(2):
                    nc.vector.tensor_copy(
                        out=cq_sb[:, m, hf * 512:(hf + 1) * 512],
                        in_=ps4[2 * m + hf][:],
                    )

            # ---- Phase B2: wqd m2/m3 + wkvd (k-inner; x resident by now) ----
            for wd, cdst, ms in (
                (wqd, cq_sb, (2, 3)),
                (wkvd, ckv_sb, (0, 1, 2, 3)),
            ):
                for m in ms:
                    wslab = wpool.tile([P, KX, P], BF, tag="wqrs", name="bslab")
                    b_src = wd[:, m * P:(m + 1) * P].rearrange(
                        "(ko p) m -> p ko m", p=P
                    )
                    nc.sync.dma_start(out=wslab[:, 0:16, :], in_=b_src[:, 0:16, :])
                    nc.sync.dma_start(out=wslab[:, 16:KX, :], in_=b_src[:, 16:KX, :])
                    for hf in range(2):
                        ps = cpp.tile([P, 512], F32, tag="cps")
                        for k in range(KX):
                            nc.tensor.matmul(
                                ps[:],
                                wslab[:, k, :],
                                xT_sb[:, k, hf * 512:(hf + 1) * 512],
                                start=(k == 0),
                                stop=(k == KX - 1),
                            )
                        nc.vector.tensor_copy(
                            out=cdst[:, m, hf * 512:(hf + 1) * 512], in_=ps[:]
                        )

            # k_rope^T [64, T]
            for hf in range(2):
                ps = cpp.tile([P, 512], F32, tag="cps")
                for k in range(KX):
                    nc.tensor.matmul(
                        ps[:DR, :],
                        wkr_sb[:, k, :],
                        xT_sb[:, k, hf * 512:(hf + 1) * 512],
                        start=(k == 0),
                        stop=(k == KX - 1),
                    )
                nc.vector.tensor_copy(
                    out=kr_sb[:, hf * 512:(hf + 1) * 512], in_=ps[:DR, :]
                )

            # ---- Phase V: v = c_kv @ Wv_up (token-major), interleaved heads ----
            for tt in range(8):
                for hf in range(2):
                    ps = cpp.tile([P, 512], F32, tag="cps")
                    for lc in range(LC):
                        nc.tensor.matmul(
                            ps[:],
                            ckv_sb[:, lc, tt * P:(tt + 1) * P],
                            wvu_sb[:, lc, hf * 512:(hf + 1) * 512],
                            start=(lc == 0),
                            stop=(lc == LC - 1),
                        )
                    nc.vector.tensor_copy(
                        out=v_sb[:, tt, hf * 16:(hf + 1) * 16, 0:32],
                        in_=ps[:].rearrange("p (h d) -> p h d", h=16),
                    )

            # ---- projections for one head group, as a list of small emission
            # steps so they can be interleaved into the previous group's
            # attention (keeps the in-order PE queue fed while exps drain).
            def proj_steps(g):
                qt = [
                    qkpool.tile([P, T], BF, tag="qkt", name=f"qt{g}_{j}")
                    for j in range(4)
                ]
                kt = [
                    qkpool.tile([P, T], BF, tag="qkt", name=f"kt{g}_{j}")
                    for j in range(4)
                ]
                steps = []
                state = {}

                # up-projections (wqu -> qt rows 64:96, wku -> kt rows 64:96)
                def up_dma(wu, key):
                    def f():
                        ws = wpool.tile([P, LC, P], BF, tag="wups", name=f"up{key}")
                        nc.sync.dma_start(
                            out=ws[:],
                            in_=wu[:, g * P:(g + 1) * P].rearrange(
                                "(c p) m -> p c m", p=P
                            ),
                        )
                        state[key] = ws
                    return f

                def up_mm(key, src, hf, lc):
                    def f():
                        if lc == 0:
                            state[(key, hf)] = cpp.tile(
                                [P, 512], F32, tag="cps", name=f"up_ps_{key}"
                            )
                        nc.tensor.matmul(
                            state[(key, hf)][:],
                            state[key][:, lc, :],
                            src[:, lc, hf * 512:(hf + 1) * 512],
                            start=(lc == 0),
                            stop=(lc == LC - 1),
                        )
                    return f

                def up_cast(key, dst, hf):
                    def f():
                        ps = state[(key, hf)]
                        sl = slice(hf * 512, (hf + 1) * 512)
                        for j in range(4):
                            nc.vector.tensor_copy(
                                out=dst[j][DR:DR + DH, sl],
                                in_=ps[j * DH:(j + 1) * DH, :],
                            )
                    return f

                for wu, key, src, dst in (
                    (wqu, "q", cq_sb, qt),
                    (wku, "k", ckv_sb, kt),
                ):
                    steps.append(up_dma(wu, key))
                    for hf in range(2):
                        for lc in range(LC):
                            steps.append(up_mm(key, src, hf, lc))
                        steps.append(up_cast(key, dst, hf))

                # shared k_rope rows into each kt (DVE, no MM deps)
                for j in range(4):
                    steps.append(
                        lambda j=j: nc.vector.tensor_copy(
                            out=kt[j][0:DR, :], in_=kr_sb[:]
                        )
                    )

                # q_rope slabs: slab s covers heads 2s, 2s+1 (rows 0:64)
                def qr_dma(s):
                    def f():
                        ws = wpool.tile([P, KX, P], BF, tag="wqrs", name=f"qr{s}")
                        src = wqr[:, (2 * g + s) * P:(2 * g + s + 1) * P].rearrange(
                            "(ko p) m -> p ko m", p=P
                        )
                        nc.sync.dma_start(out=ws[:, 0:16, :], in_=src[:, 0:16, :])
                        nc.sync.dma_start(out=ws[:, 16:KX, :], in_=src[:, 16:KX, :])
                        state[("qr", s)] = ws
                    return f

                def qr_mm(s, hf, k):
                    def f():
                        if k == 0:
                            state[("qrps", s, hf)] = cpp.tile(
                                [P, 512], F32, tag="cps", name="qr_ps"
                            )
                        nc.tensor.matmul(
                            state[("qrps", s, hf)][:],
                            state[("qr", s)][:, k, :],
                            xT_sb[:, k, hf * 512:(hf + 1) * 512],
                            start=(k == 0),
                            stop=(k == KX - 1),
                        )
                    return f

                def qr_cast(s, hf):
                    def f():
                        ps = state[("qrps", s, hf)]
                        sl = slice(hf * 512, (hf + 1) * 512)
                        nc.vector.tensor_copy(out=qt[2 * s][0:DR, sl], in_=ps[0:DR, :])
                        nc.vector.tensor_copy(
                            out=qt[2 * s + 1][0:DR, sl], in_=ps[DR:P, :]
                        )
                    return f

                for s in range(2):
                    steps.append(qr_dma(s))
                    for hf in range(2):
                        for k in range(KX):
                            steps.append(qr_mm(s, hf, k))
                        steps.append(qr_cast(s, hf))
                return qt, kt, steps

            # ---- attention for group g, with proj(g+1) steps interleaved ----
            def emit_attn(g, qt, kt, steps):
                si = 0
                nsteps = len(steps)

                def fill(n, limit):
                    nonlocal si
                    end = min(limit, si + n, nsteps)
                    while si < end:
                        steps[si]()
                        si += 1

                chains = [(j, qc) for qc in range(2) for j in range(4)]
                nch = len(chains)
                # per-group batched softmax denominators: DVE reciprocal cost
                # scales with per-partition elements, so spread the 8 ones-row
                # results across partitions and run 2 reciprocals per group
                # instead of 8 single-partition ones (8x less DVE time).
                avs_list = []

                def normalize(ci):
                    j, qsl = avs_list[ci]
                    # rrep band matches aout's base partition (TensorTensor
                    # requires equal SBUF base partitions on both inputs).
                    # partition_broadcast replicates PARTITION 0 of its input
                    # and writes from partition 0 — both APs' bases are
                    # ignored — so stage the reciprocal row to partition 0
                    # (bf16: cheaper broadcast, precision is bf16 anyway).
                    rrep = rrpool.tile([P, 512], BF, tag="rr", name="rrep")
                    rst = rrpool.tile([1, 512], BF, tag="rst", name="rst")
                    dp = (ci % 4) * 32
                    jb = j * DH
                    nc.vector.tensor_copy(
                        out=rst[:], in_=recs[ci // 4][dp:dp + 1, :]
                    )
                    nc.gpsimd.partition_broadcast(rrep[:], rst[:])
                    nc.vector.tensor_mul(
                        out=aout_sb[jb:jb + DH, g, qsl],
                        in0=aout_sb[jb:jb + DH, g, qsl],
                        in1=rrep[jb:jb + DH, :],
                    )

                for ci, (j, qc) in enumerate(chains):
                    limit = (nsteps * (ci + 1) + nch - 1) // nch
                    h = 4 * g + j
                    qsl = slice(qc * 512, (qc + 1) * 512)
                    probs = ppool.tile([P, 8, 512], BF, tag="probs", name="probs")
                    for kc in range(8):
                        sp = spp.tile([P, 512], F32, tag="sps", name="sps")
                        nc.tensor.matmul(
                            sp[:],
                            kt[j][0:96, kc * P:(kc + 1) * P],
                            qt[j][0:96, qsl],
                            start=True,
                            stop=True,
                        )
                        nc.scalar.activation(
                            out=probs[:, kc, :],
                            in_=sp[:],
                            func=mybir.ActivationFunctionType.Exp,
                            scale=SCALE,
                        )
                        if kc >= 1:
                            fill(3, limit)
                    av = avp.tile([33, 512], F32, tag="avp", name="av")
                    for kc in range(8):
                        nc.tensor.matmul(
                            av[:],
                            v_sb[:, kc, h, 0:33],
                            probs[:, kc, :],
                            start=(kc == 0),
                            stop=(kc == 7),
                        )
                        if kc % 2 == 1:
                            fill(2, limit)
                    # stage unnormalized values straight into aout (bf16),
                    # normalize in place after the batched reciprocal
                    nc.vector.tensor_copy(
                        out=aout_sb[j * DH:(j + 1) * DH, g, qsl],
                        in_=av[0:DH, :],
                    )
                    dp = (ci % 4) * 32
                    nc.vector.tensor_copy(
                        out=dens[ci // 4][dp:dp + 1, :], in_=av[32:33, :]
                    )
                    avs_list.append((j, qsl))
                    fill(nsteps, limit)
                    if ci == 3:
                        # first half's denominators complete: reciprocal now,
                        # normalize chains 0-3 spread across chains 4-7 so the
                        # group-end DVE burst (which head-of-line blocks the
                        # next group's proj casts) is halved
                        nc.vector.reciprocal(recs[0][:], dens[0][:])
                    elif ci > 3:
                        normalize(ci - 4)
                nc.vector.reciprocal(recs[1][:], dens[1][:])
                for ci in range(4, 8):
                    normalize(ci)
                fill(nsteps, nsteps)

            qt, kt, steps0 = proj_steps(0)
            for st in steps0:
                st()
            for g in range(8):
                if g < 7:
                    nqt, nkt, steps = proj_steps(g + 1)
                else:
                    steps = []
                if g == 6:
                    nc.sync.dma_start(
                        out=wos0_sb[:],
                        in_=wo[:, 0:512].rearrange("(kc p) m -> p kc m", p=P)[
                            :, 0:2, :
                        ],
                    )
                emit_attn(g, qt, kt, steps)
                if g < 7:
                    qt, kt = nqt, nkt

        # ---- Phase E: out = aout^T @ Wo  (token-major), Wo streamed once ----
        with ExitStack() as ctx:
            wop = ctx.enter_context(tc.tile_pool(name="wop", bufs=2))
            epp = ctx.enter_context(tc.tile_pool(name="epp", bufs=8, space="PSUM"))
            eop = ctx.enter_context(tc.tile_pool(name="eop", bufs=2))
            for n in range(8):
                wo_src = wo[:, n * 512:(n + 1) * 512].rearrange(
                    "(kc p) m -> p kc m", p=P
                )
                woslab = wop.tile([P, 8, 512], BF, tag="wos")
                if n == 0:
                    # first two kc chunks prefetched during group-6 attention
                    nc.sync.dma_start(out=woslab[:, 2:8, :], in_=wo_src[:, 2:8, :])
                else:
                    nc.sync.dma_start(out=woslab[:, 0:4, :], in_=wo_src[:, 0:4, :])
                    nc.sync.dma_start(out=woslab[:, 4:8, :], in_=wo_src[:, 4:8, :])
                pss = [
                    epp.tile([P, 512], F32, tag="eps", name=f"eps_{n}_{i}")
                    for i in range(8)
                ]
                # batched output tile: one store DMA per n instead of 8
                # (the Sync engine's ~1us per descriptor was pacing phase E);
                # per-tt copies right after each tt's last matmul so PSUM
                # slots free progressively before n+1 starts
                eot = eop.tile([P, 8, 512], F32, tag="eot")
                for kc in range(8):
                    if n == 0 and kc < 2:
                        src = wos0_sb[:, kc, :]
                    else:
                        src = woslab[:, kc, :]
                    for tt in range(8):
                        nc.tensor.matmul(
                            pss[tt][:],
                            aout_sb[:, kc, tt * P:(tt + 1) * P],
                            src,
                            start=(kc == 0),
                            stop=(kc == 7),
                        )
                        if kc == 7:
                            nc.vector.tensor_copy(
                                out=eot[:, tt, :], in_=pss[tt][:]
                            )
                nc.sync.dma_start(
                    out=out[:, n * 512:(n + 1) * 512].rearrange(
                        "(tt p) m -> p tt m", p=P
                    ),
                    in_=eot[:],
                )

    nc.compile()
    return nc


def _prep_inputs(inputs):
    bf = ml_dtypes.bfloat16
    x = np.asarray(inputs["x"], dtype=np.float32)
    Wq_down = np.asarray(inputs["Wq_down"], dtype=np.float32).astype(bf)
    Wkv_down = np.asarray(inputs["Wkv_down"], dtype=np.float32).astype(bf)
    Wq_up = np.asarray(inputs["Wq_up"], dtype=np.float32).astype(bf)
    Wk_up = np.asarray(inputs["Wk_up"], dtype=np.float32).astype(bf)
    Wv_up = np.asarray(inputs["Wv_up"], dtype=np.float32).astype(bf)
    Wq_rope = np.asarray(inputs["Wq_rope"], dtype=np.float32).astype(bf)
    Wk_rope = np.asarray(inputs["Wk_rope"], dtype=np.float32).astype(bf)
    Wo = np.asarray(inputs["Wo"], dtype=np.float32).astype(bf)

    xT = [np.ascontiguousarray(x[b].T).astype(bf) for b in range(NB)]

    in_maps = []
    for core in range(8):
        b = core // 4
        hg = core % 4
        hs = slice(hg * NHC * DH, (hg + 1) * NHC * DH)        # head-dim cols
        rs = slice(hg * NHC * DR, (hg + 1) * NHC * DR)        # rope cols
        in_maps.append(
            {
                "xT": xT[b],
                "wqd": Wq_down,
                "wkvd": Wkv_down,
                "wqu": np.ascontiguousarray(Wq_up[:, hs]),
                "wku": np.ascontiguousarray(Wk_up[:, hs]),
                "wvu": np.ascontiguousarray(Wv_up[:, hs]),
                "wqr": np.ascontiguousarray(Wq_rope[:, rs]),
                "wkr": Wk_rope,
                "wo": np.ascontiguousarray(Wo[hs, :]),
            }
        )
    return in_maps


def kernel(**inputs):
    if "nc" not in _CACHE:
        _CACHE["nc"] = _build_program()
    nc = _CACHE["nc"]
    in_maps = _prep_inputs(inputs)
    res = run_bass_kernel_spmd(nc, in_maps, list(range(8)))
    out = np.zeros((NB, T, DM), dtype=np.float32)
    for core in range(8):
        out[core // 4] += res.results[core]["out"]
    return out


# revision 36
# speedup vs baseline: 1.1871x; 1.1473x over previous
"""MLA attention kernel for 8 Trainium2 NeuronCores.

Sharding: core i -> batch b = i//4, head group hg = i%4 (32 heads each).
Latent down-projections replicated within a batch group; Wq_up/Wq_rope/
Wk_up/Wv_up/Wo sharded by head.  Host sums the 4 partial outputs per batch.

Device program (identical on all cores, SPMD over different data):
  - all matmuls bf16 with fp32 PSUM accumulation
  - projections computed feature-major (features on partitions) so that
    attention scores S^T[k, q] = kT.T @ qT need no transposes
  - softmax: exp on ScalarE (scale 1/sqrt(96) folded in, no max subtraction:
    scores are ~N(0,1)), denominator via an appended ones-column of V in the
    attn@V matmul, division via DVE reciprocal_approx_fast + gpsimd
    partition broadcast
  - startup: first two Wq_down m-slabs stream in, then x; the down-proj
    runs k-OUTER over 4 PSUM accumulators so the PE consumes x chunks as
    the 8MB DMA delivers them (instead of stalling ~38us for all of x)
  - the attention exp chains are ACT-throughput bound (8 exps x 687ns vs
    3.4us of PE work per (head,qc) chain); proj MMs for group g+1 are
    emitted INTERLEAVED between attn(g)'s score/av matmuls so the strict
    in-order PE queue always has independent work while exps drain
"""

import sys

sys.path.insert(0, "/opt/trn_rl_repo")

import numpy as np
import ml_dtypes

import concourse.bass as bass
import concourse.tile as tile
from concourse import bacc, mybir
from concourse.bass_utils import run_bass_kernel_spmd

P = 128
T = 1024          # tokens per batch
DM = 4096         # d_model
KX = DM // P      # 32 feature chunks of x
LAT = 512         # latent dim
LC = LAT // P     # 4 latent chunks
NHC = 32          # heads per core
DH = 32           # head dim (compressed part)
DR = 64           # rope dim per head
NB = 2            # batch
SCALE = 1.0 / float(np.sqrt(DH + DR))

BF = mybir.dt.bfloat16
F32 = mybir.dt.float32

_CACHE = {}


def _build_program():
    nc = bacc.Bacc("TRN2", target_bir_lowering=False, num_devices=8)

    xT = nc.declare_dram_parameter("xT", [DM, T], BF, isOutput=False)
    wqd = nc.declare_dram_parameter("wqd", [DM, LAT], BF, isOutput=False)
    wkvd = nc.declare_dram_parameter("wkvd", [DM, LAT], BF, isOutput=False)
    wqu = nc.declare_dram_parameter("wqu", [LAT, NHC * DH], BF, isOutput=False)
    wku = nc.declare_dram_parameter("wku", [LAT, NHC * DH], BF, isOutput=False)
    wvu = nc.declare_dram_parameter("wvu", [LAT, NHC * DH], BF, isOutput=False)
    wqr = nc.declare_dram_parameter("wqr", [DM, NHC * DR], BF, isOutput=False)
    wkr = nc.declare_dram_parameter("wkr", [DM, DR], BF, isOutput=False)
    wo = nc.declare_dram_parameter("wo", [NHC * DH, DM], BF, isOutput=False)
    out = nc.declare_dram_parameter("out", [T, DM], F32, isOutput=True)

    from contextlib import ExitStack

    with tile.TileContext(nc) as tc, ExitStack() as octx:
        const = octx.enter_context(tc.tile_pool(name="const", bufs=1))

        xT_sb = const.tile([P, KX, T], BF, name="xT_sb")
        xT_r = xT[:].rearrange("(ko p) t -> p ko t", p=P)
        wkr_sb = const.tile([P, KX, DR], BF, name="wkr_sb")
        wvu_sb = const.tile([P, LC, NHC * DH], BF, name="wvu_sb")

        cq_sb = const.tile([P, LC, T], BF, name="cq_sb")      # c_q^T
        ckv_sb = const.tile([P, LC, T], BF, name="ckv_sb")    # c_kv^T
        kr_sb = const.tile([DR, T], BF, name="kr_sb")         # k_rope^T (shared)
        # v token-major, per (key-chunk, head): cols 0:32 = v, col 32 = ones
        v_sb = const.tile([P, 8, NHC, 34], BF, name="v_sb")
        # attention output, feature-major: head h -> [32*(h%4):.., h//4, :]
        aout_sb = const.tile([P, 8, T], BF, name="aout_sb")
        # prefetch buffer for the first half of Wo slab n=0 (removes the
        # phase-E start stall; the rest double-buffers under E compute)
        wos0_sb = const.tile([P, 2, 512], BF, name="wos0_sb")
        # softmax denominators, spread across partitions {0,32,64,96} of two
        # tiles (DVE writes must be 32-partition-aligned); persistent+memset
        # so the batched reciprocal never reads uninitialized rows
        dens = [const.tile([P, 512], F32, name=f"den{i}") for i in range(2)]
        recs = [const.tile([P, 512], F32, name=f"rec{i}") for i in range(2)]

        nc.vector.memset(v_sb[:, :, :, 32:33], 1.0)
        for i in range(2):
            nc.vector.memset(dens[i][:], 1.0)

        with ExitStack() as ctx:
            wpool = ctx.enter_context(tc.tile_pool(name="wpool", bufs=3))
            cpp = ctx.enter_context(tc.tile_pool(name="cpp", bufs=4, space="PSUM"))
            qkpool = ctx.enter_context(tc.tile_pool(name="qkpool", bufs=12))
            ppool = ctx.enter_context(tc.tile_pool(name="ppool", bufs=2))
            spp = ctx.enter_context(tc.tile_pool(name="spp", bufs=2, space="PSUM"))
            avp = ctx.enter_context(tc.tile_pool(name="avp", bufs=2, space="PSUM"))
            rrpool = ctx.enter_context(tc.tile_pool(name="rrpool", bufs=1))

            # ---- DMA order: first two wqd slabs, then x (8 fine chunks so
            # the k-outer loop can start early), then wkr/wvu.
            def load_bslab(m):
                ws = wpool.tile([P, KX, P], BF, tag="wqrs", name=f"bq{m}")
                b_src = wqd[:, m * P:(m + 1) * P].rearrange(
                    "(ko p) m -> p ko m", p=P
                )
                nc.sync.dma_start(out=ws[:, 0:16, :], in_=b_src[:, 0:16, :])
                nc.sync.dma_start(out=ws[:, 16:KX, :], in_=b_src[:, 16:KX, :])
                return ws

            def load_x_quad(i):
                nc.sync.dma_start(
                    out=xT_sb[:, i * 4:(i + 1) * 4, :],
                    in_=xT_r[:, i * 4:(i + 1) * 4, :],
                )

            # interleave weight-slab and x DMAs so neither gates the other
            bslab01 = [load_bslab(0)]
            load_x_quad(0)
            bslab01.append(load_bslab(1))
            for i in range(1, 8):
                load_x_quad(i)
            nc.sync.dma_start(
                out=wkr_sb[:], in_=wkr[:].rearrange("(ko p) d -> p ko d", p=P)
            )
            nc.sync.dma_start(
                out=wvu_sb[:], in_=wvu[:].rearrange("(c p) m -> p c m", p=P)
            )

            # ---- Phase B1: wqd m0/m1, k-OUTER over 4 accumulators so the
            # PE consumes x chunks at DMA pace instead of stalling; blocks
            # of 4 k alternating m so the m1 chains don't head-of-line block
            # before the m1 slab DMA lands.
            ps4 = [
                cpp.tile([P, 512], F32, tag="cps", name=f"b01_{i}")
                for i in range(4)
            ]
            for kb in range(8):
                for m in range(2):
                    for k in range(kb * 4, kb * 4 + 4):
                        for hf in range(2):
                            nc.tensor.matmul(
                                ps4[2 * m + hf][:],
                                bslab01[m][:, k, :],
                                xT_sb[:, k, hf * 512:(hf + 1) * 512],
                                start=(k == 0),
                                stop=(k == KX - 1),
                            )
            for m in range(2):
                for hf in range(2):
                    nc.vector.tensor_copy(
                        out=cq_sb[:, m, hf * 512:(hf + 1) * 512],
                        in_=ps4[2 * m + hf][:],
                    )

            # ---- Phase B2: wqd m2/m3 + wkvd (k-inner; x resident by now) ----
            for wd, cdst, ms in (
                (wqd, cq_sb, (2, 3)),
                (wkvd, ckv_sb, (0, 1, 2, 3)),
            ):
                for m in ms:
                    wslab = wpool.tile([P, KX, P], BF, tag="wqrs", name="bslab")
                    b_src = wd[:, m * P:(m + 1) * P].rearrange(
                        "(ko p) m -> p ko m", p=P
                    )
                    nc.sync.dma_start(out=wslab[:, 0:16, :], in_=b_src[:, 0:16, :])
                    nc.sync.dma_start(out=wslab[:, 16:KX, :], in_=b_src[:, 16:KX, :])
                    for hf in range(2):
                        ps = cpp.tile([P, 512], F32, tag="cps")
                        for k in range(KX):
                            nc.tensor.matmul(
                                ps[:],
                                wslab[:, k, :],
                                xT_sb[:, k, hf * 512:(hf + 1) * 512],
                                start=(k == 0),
                                stop=(k == KX - 1),
                            )
                        nc.vector.tensor_copy(
                            out=cdst[:, m, hf * 512:(hf + 1) * 512], in_=ps[:]
                        )

            # k_rope^T [64, T]
            for hf in range(2):
                ps = cpp.tile([P, 512], F32, tag="cps")
                for k in range(KX):
                    nc.tensor.matmul(
                        ps[:DR, :],
                        wkr_sb[:, k, :],
                        xT_sb[:, k, hf * 512:(hf + 1) * 512],
                        start=(k == 0),
                        stop=(k == KX - 1),
                    )
                nc.vector.tensor_copy(
                    out=kr_sb[:, hf * 512:(hf + 1) * 512], in_=ps[:DR, :]
                )

            # ---- Phase V: v = c_kv @ Wv_up (token-major), interleaved heads ----
            for tt in range(8):
                for hf in range(2):
                    ps = cpp.tile([P, 512], F32, tag="cps")
                    for lc in range(LC):
                        nc.tensor.matmul(
                            ps[:],
                            ckv_sb[:, lc, tt * P:(tt + 1) * P],
                            wvu_sb[:, lc, hf * 512:(hf + 1) * 512],
                            start=(lc == 0),
                            stop=(lc == LC - 1),
                        )
                    nc.vector.tensor_copy(
                        out=v_sb[:, tt, hf * 16:(hf + 1) * 16, 0:32],
                        in_=ps[:].rearrange("p (h d) -> p h d", h=16),
                    )

            # ---- projections for one head group, as a list of small emission
            # steps so they can be interleaved into the previous group's
            # attention (keeps the in-order PE queue fed while exps drain).
            def proj_steps(g):
                qt = [
                    qkpool.tile([P, T], BF, tag="qkt", name=f"qt{g}_{j}")
                    for j in range(4)
                ]
                kt = [
                    qkpool.tile([P, T], BF, tag="qkt", name=f"kt{g}_{j}")
                    for j in range(4)
                ]
                steps = []
                state = {}

                # up-projections (wqu -> qt rows 64:96, wku -> kt rows 64:96)
                def up_dma(wu, key):
                    def f():
                        ws = wpool.tile([P, LC, P], BF, tag="wups", name=f"up{key}")
                        nc.sync.dma_start(
                            out=ws[:],
                            in_=wu[:, g * P:(g + 1) * P].rearrange(
                                "(c p) m -> p c m", p=P
                            ),
                        )
                        state[key] = ws
                    return f

                def up_mm(key, src, hf, lc):
                    def f():
                        if lc == 0:
                            state[(key, hf)] = cpp.tile(
                                [P, 512], F32, tag="cps", name=f"up_ps_{key}"
                            )
                        nc.tensor.matmul(
                            state[(key, hf)][:],
                            state[key][:, lc, :],
                            src[:, lc, hf * 512:(hf + 1) * 512],
                            start=(lc == 0),
                            stop=(lc == LC - 1),
                        )
                    return f

                def up_cast(key, dst, hf):
                    def f():
                        ps = state[(key, hf)]
                        sl = slice(hf * 512, (hf + 1) * 512)
                        for j in range(4):
                            nc.vector.tensor_copy(
                                out=dst[j][DR:DR + DH, sl],
                                in_=ps[j * DH:(j + 1) * DH, :],
                            )
                    return f

                for wu, key, src, dst in (
                    (wqu, "q", cq_sb, qt),
                    (wku, "k", ckv_sb, kt),
                ):
                    steps.append(up_dma(wu, key))
                    for hf in range(2):
                        for lc in range(LC):
                            steps.append(up_mm(key, src, hf, lc))
                        steps.append(up_cast(key, dst, hf))

                # shared k_rope rows into each kt (DVE, no MM deps)
                for j in range(4):
                    steps.append(
                        lambda j=j: nc.vector.tensor_copy(
                            out=kt[j][0:DR, :], in_=kr_sb[:]
                        )
                    )

                # q_rope slabs: slab s covers heads 2s, 2s+1 (rows 0:64)
                def qr_dma(s):
                    def f():
                        ws = wpool.tile([P, KX, P], BF, tag="wqrs", name=f"qr{s}")
                        src = wqr[:, (2 * g + s) * P:(2 * g + s + 1) * P].rearrange(
                            "(ko p) m -> p ko m", p=P
                        )
                        nc.sync.dma_start(out=ws[:, 0:16, :], in_=src[:, 0:16, :])
                        nc.sync.dma_start(out=ws[:, 16:KX, :], in_=src[:, 16:KX, :])
                        state[("qr", s)] = ws
                    return f

                def qr_mm(s, hf, k):
                    def f():
                        if k == 0:
                            state[("qrps", s, hf)] = cpp.tile(
                                [P, 512], F32, tag="cps", name="qr_ps"
                            )
                        nc.tensor.matmul(
                            state[("qrps", s, hf)][:],
                            state[("qr", s)][:, k, :],
                            xT_sb[:, k, hf * 512:(hf + 1) * 512],
                            start=(k == 0),
                            stop=(k == KX - 1),
                        )
                    return f

                def qr_cast(s, hf):
                    def f():
                        ps = state[("qrps", s, hf)]
                        sl = slice(hf * 512, (hf + 1) * 512)
                        nc.vector.tensor_copy(out=qt[2 * s][0:DR, sl], in_=ps[0:DR, :])
                        nc.vector.tensor_copy(
                            out=qt[2 * s + 1][0:DR, sl], in_=ps[DR:P, :]
                        )
                    return f

                for s in range(2):
                    steps.append(qr_dma(s))
                    for hf in range(2):
                        for k in range(KX):
                            steps.append(qr_mm(s, hf, k))
                        steps.append(qr_cast(s, hf))
                return qt, kt, steps

            # ---- attention for group g, with proj(g+1) steps interleaved ----
            def emit_attn(g, qt, kt, steps):
                si = 0
                nsteps = len(steps)

                def fill(n, limit):
                    nonlocal si
                    end = min(limit, si + n, nsteps)
                    while si < end:
                        steps[si]()
                        si += 1

                chains = [(j, qc) for qc in range(2) for j in range(4)]
                nch = len(chains)
                # per-group batched softmax denominators: DVE reciprocal cost
                # scales with per-partition elements, so spread the 8 ones-row
                # results across partitions and run 2 reciprocals per group
                # instead of 8 single-partition ones (8x less DVE time).
                avs_list = []

                def normalize(ci):
                    j, qsl = avs_list[ci]
                    # rrep band matches aout's base partition (TensorTensor
                    # requires equal SBUF base partitions on both inputs).
                    # partition_broadcast replicates PARTITION 0 of its input
                    # and writes from partition 0 — both APs' bases are
                    # ignored — so stage the reciprocal row to partition 0
                    # (bf16: cheaper broadcast, precision is bf16 anyway).
                    rrep = rrpool.tile([P, 512], BF, tag="rr", name="rrep")
                    rst = rrpool.tile([1, 512], BF, tag="rst", name="rst")
                    dp = (ci % 4) * 32
                    jb = j * DH
                    nc.vector.tensor_copy(
                        out=rst[:], in_=recs[ci // 4][dp:dp + 1, :]
                    )
                    nc.gpsimd.partition_broadcast(rrep[:], rst[:])
                    nc.vector.tensor_mul(
                        out=aout_sb[jb:jb + DH, g, qsl],
                        in0=aout_sb[jb:jb + DH, g, qsl],
                        in1=rrep[jb:jb + DH, :],
                    )

                for ci, (j, qc) in enumerate(chains):
                    limit = (nsteps * (ci + 1) + nch - 1) // nch
                    h = 4 * g + j
                    qsl = slice(qc * 512, (qc + 1) * 512)
                    probs = ppool.tile([P, 8, 512], BF, tag="probs", name="probs")
                    for kc in range(8):
                        sp = spp.tile([P, 512], F32, tag="sps", name="sps")
                        nc.tensor.matmul(
                            sp[:],
                            kt[j][0:96, kc * P:(kc + 1) * P],
                            qt[j][0:96, qsl],
                            start=True,
                            stop=True,
                        )
                        nc.scalar.activation(
                            out=probs[:, kc, :],
                            in_=sp[:],
                            func=mybir.ActivationFunctionType.Exp,
                            scale=SCALE,
                        )
                        if kc >= 1:
                            fill(3, limit)
                    av = avp.tile([33, 512], F32, tag="avp", name="av")
                    for kc in range(8):
                        nc.tensor.matmul(
                            av[:],
                            v_sb[:, kc, h, 0:33],
                            probs[:, kc, :],
                            start=(kc == 0),
                            stop=(kc == 7),
                        )
                        if kc % 2 == 1:
                            fill(2, limit)
                    # stage unnormalized values straight into aout (bf16),
                    # normalize in place after the batched reciprocal
                    nc.vector.tensor_copy(
                        out=aout_sb[j * DH:(j + 1) * DH, g, qsl],
                        in_=av[0:DH, :],
                    )
                    dp = (ci % 4) * 32
                    nc.vector.tensor_copy(
                        out=dens[ci // 4][dp:dp + 1, :], in_=av[32:33, :]
                    )
                    avs_list.append((j, qsl))
                    fill(nsteps, limit)
                    if ci == 3:
                        # first half's denominators complete: reciprocal now,
                        # normalize chains 0-3 spread across chains 4-7 so the
                        # group-end DVE burst (which head-of-line blocks the
                        # next group's proj casts) is halved
                        nc.vector.reciprocal(recs[0][:], dens[0][:])
                    elif ci > 3:
                        normalize(ci - 4)
                nc.vector.reciprocal(recs[1][:], dens[1][:])
                for ci in range(4, 8):
                    normalize(ci)
                fill(nsteps, nsteps)

            qt, kt, steps0 = proj_steps(0)
            for st in steps0:
                st()
            for g in range(8):
                if g < 7:
                    nqt, nkt, steps = proj_steps(g + 1)
                else:
                    steps = []
                if g == 6:
                    nc.sync.dma_start(
                        out=wos0_sb[:],
                        in_=wo[:, 0:512].rearrange("(kc p) m -> p kc m", p=P)[
                            :, 0:2, :
                        ],
                    )
                emit_attn(g, qt, kt, steps)
                if g < 7:
                    qt, kt = nqt, nkt

        # ---- Phase E: out = aout^T @ Wo  (token-major), Wo streamed once ----
        with ExitStack() as ctx:
            wop = ctx.enter_context(tc.tile_pool(name="wop", bufs=2))
            epp = ctx.enter_context(tc.tile_pool(name="epp", bufs=8, space="PSUM"))
            eop = ctx.enter_context(tc.tile_pool(name="eop", bufs=2))
            for n in range(8):
                wo_src = wo[:, n * 512:(n + 1) * 512].rearrange(
                    "(kc p) m -> p kc m", p=P
                )
                woslab = wop.tile([P, 8, 512], BF, tag="wos")
                if n == 0:
                    # first two kc chunks prefetched during group-6 attention
                    nc.sync.dma_start(out=woslab[:, 2:8, :], in_=wo_src[:, 2:8, :])
                else:
                    nc.sync.dma_start(out=woslab[:, 0:4, :], in_=wo_src[:, 0:4, :])
                    nc.sync.dma_start(out=woslab[:, 4:8, :], in_=wo_src[:, 4:8, :])
                pss = [
                    epp.tile([P, 512], F32, tag="eps", name=f"eps_{n}_{i}")
                    for i in range(8)
                ]
                # batched output tile: one store DMA per n instead of 8
                # (the Sync engine's ~1us per descriptor was pacing phase E);
                # per-tt copies right after each tt's last matmul so PSUM
                # slots free progressively before n+1 starts
                eot = eop.tile([P, 8, 512], F32, tag="eot")
                for kc in range(8):
                    if n == 0 and kc < 2:
                        src = wos0_sb[:, kc, :]
                    else:
                        src = woslab[:, kc, :]
                    for tt in range(8):
                        nc.tensor.matmul(
                            pss[tt][:],
                            aout_sb[:, kc, tt * P:(tt + 1) * P],
                            src,
                            start=(kc == 0),
                            stop=(kc == 7),
                        )
                        if kc == 7:
                            nc.vector.tensor_copy(
                                out=eot[:, tt, :], in_=pss[tt][:]
                            )
                nc.sync.dma_start(
                    out=out[:, n * 512:(n + 1) * 512].rearrange(
                        "(tt p) m -> p tt m", p=P
                    ),
                    in_=eot[:],
                )

    nc.compile()
    return nc


def _prep_inputs(inputs):
    bf = ml_dtypes.bfloat16
    x = np.asarray(inputs["x"], dtype=np.float32)
    Wq_down = np.asarray(inputs["Wq_down"], dtype=np.float32).astype(bf)
    Wkv_down = np.asarray(inputs["Wkv_down"], dtype=np.float32).astype(bf)
    Wq_up = np.asarray(inputs["Wq_up"], dtype=np.float32).astype(bf)
    Wk_up = np.asarray(inputs["Wk_up"], dtype=np.float32).astype(bf)
    Wv_up = np.asarray(inputs["Wv_up"], dtype=np.float32).astype(bf)
    Wq_rope = np.asarray(inputs["Wq_rope"], dtype=np.float32).astype(bf)
    Wk_rope = np.asarray(inputs["Wk_rope"], dtype=np.float32).astype(bf)
    Wo = np.asarray(inputs["Wo"], dtype=np.float32).astype(bf)

    xT = [np.ascontiguousarray(x[b].T).astype(bf) for b in range(NB)]

    in_maps = []
    for core in range(8):
        b = core // 4
        hg = core % 4
        hs = slice(hg * NHC * DH, (hg + 1) * NHC * DH)        # head-dim cols
        rs = slice(hg * NHC * DR, (hg + 1) * NHC * DR)        # rope cols
        in_maps.append(
            {
                "xT": xT[b],
                "wqd": Wq_down,
                "wkvd": Wkv_down,
                "wqu": np.ascontiguousarray(Wq_up[:, hs]),
                "wku": np.ascontiguousarray(Wk_up[:, hs]),
                "wvu": np.ascontiguousarray(Wv_up[:, hs]),
                "wqr": np.ascontiguousarray(Wq_rope[:, rs]),
                "wkr": Wk_rope,
                "wo": np.ascontiguousarray(Wo[hs, :]),
            }
        )
    return in_maps


def kernel(**inputs):
    if "nc" not in _CACHE:
        _CACHE["nc"] = _build_program()
    nc = _CACHE["nc"]
    in_maps = _prep_inputs(inputs)
    res = run_bass_kernel_spmd(nc, in_maps, list(range(8)))
    out = np.zeros((NB, T, DM), dtype=np.float32)
    for core in range(8):
        out[core // 4] += res.results[core]["out"]
    return out
